# revision 1
# baseline (speedup 1.0000x reference)
"""ConvDeepSet kernel for Trainium2 (8 NeuronCores, Bass/Tile).

Math (per batch b, target point o, channel c):
    agg[o,c] = sum_i yd[i,c] * exp(-alpha_c * (x_i - t_o)^2)      yd = [1 | y]
    out[o,:] = [agg0, agg1/(agg0+eps), ...] @ W + b

Channels sharing a sigma value share the RBF matrix E[i,o], so with G
sigma-groups the aggregation collapses to G matmuls after folding W into
the context values on the host:
    U_g = sum_{c in g, c>0} yd[:,c] W[c,:]              (n_in, 16)
    P[o, 0]    = den[o]  = sum_i E_g0[i,o]              (density group g0)
    P[o, 1+j] += sum_i E_g[i,o] U_g[i,j]                (all groups)
    out[o,:]  = den*W[0,:] + P[o,1:]/(den+eps) + b

The exponent -a(x-t)^2 = s*u - s^2/2 - u^2/2 with s = sqrt(2a)x, u =
sqrt(2a)t is a rank-8 fp16 matmul using 2-way fp16 splits of s, u and of
q = s^2/2, w = u^2/2 (host-prepared), exact to ~1e-4 absolute — fp32
quality at fp16 matmul speed (single pass + fast weight load). ScalarE
applies exp (PSUM -> SBUF, fp16 out); the aggregation runs in fp16 with
fp32 PSUM accumulation (max abs output error ~5e-4 on the reference data,
~5e-5 of the output scale).

Sharding: core c -> (batch c//2, output half c%2). Per-core data all
lives in SBUF; the (n_in, n_out, C) intermediate never materializes.
"""

import numpy as np

B, N_IN, N_OUT = 4, 1024, 1024
IN_CH_RAW, OUT_CH = 7, 16
IN_CH = IN_CH_RAW + 1
N_CORES = 8
O_CORE = N_OUT // 2          # 512 target points per core
P = 128                      # partitions
KI = N_IN // P               # 8 contraction chunks
NPAIR = KI // 2              # exp processed in chunk pairs
KJ = O_CORE // P             # 4 output chunks
KEXP = 8                     # rows of the split-product exponent matmul
EPS = 1e-8

_BASS_CACHE: dict = {}


def _build_fp16_raw(widths):
    """Raw-Bass (no Tile) version of the fp16 pipeline: hand-rolled
    semaphores, no Tile entry/exit barrier stages — saves ~10us of
    framework overhead at these kernel sizes."""
    import concourse.bass as bass
    from concourse import mybir

    f32 = mybir.dt.float32
    f16 = mybir.dt.float16
    G = len(widths)
    wtot = sum(widths)
    offs = np.cumsum([0] + list(widths))
    npair = NPAIR * G
    Exp = mybir.ActivationFunctionType.Exp

    nc = bass.Bass("TRN2", target_bir_lowering=False, debug=False)

    # xr = per-group [lxh | rth] blocks along the free dim (groups must sit at
    # partition 0 for PE); one DMA feeds the first matmul. wbb = [wb0 | bt].
    BLK = N_IN + O_CORE
    xr_d = nc.dram_tensor("xr", [KEXP, G * BLK], f16, kind="ExternalInput")
    ydt_d = nc.dram_tensor("ydt", [P, KI * wtot], f16, kind="ExternalInput")
    wbb_d = nc.dram_tensor("wbb", [P, 2 * OUT_CH], f32, kind="ExternalInput")
    out_d = nc.dram_tensor("out", [O_CORE, OUT_CH], f32, kind="ExternalOutput")

    xr = nc.alloc_sbuf_tensor("xr_sb", [KEXP, G * BLK], f16).ap()
    lxh = [xr[:, g * BLK:g * BLK + N_IN] for g in range(G)]
    rth = [xr[:, g * BLK + N_IN:(g + 1) * BLK] for g in range(G)]
    ydt = nc.alloc_sbuf_tensor("ydt_sb", [P, KI * wtot], f16).ap()
    wbb = nc.alloc_sbuf_tensor("wbb_sb", [P, 2 * OUT_CH], f32).ap()
    wb0 = wbb[:, :OUT_CH]
    bt = wbb[:, OUT_CH:]
    warm = nc.alloc_sbuf_tensor("warm_sb", [1, 1], f32).ap()
    E = [nc.alloc_sbuf_tensor(f"e_sb{q}", [P, 2 * O_CORE], f16).ap()
         for q in range(npair)]
    denp = [nc.alloc_sbuf_tensor(f"denp_sb{kj}", [P, 1], f32).ap()
            for kj in range(KJ)]
    recip = [nc.alloc_sbuf_tensor(f"recip_sb{kj}", [P, 1], f32).ap()
             for kj in range(KJ)]
    t1 = [nc.alloc_sbuf_tensor(f"t1_sb{kj}", [P, OUT_CH], f32).ap()
          for kj in range(KJ)]
    o_all = nc.alloc_sbuf_tensor("o_all_sb", [P, KJ * OUT_CH], f32).ap()
    o_sb = [o_all[:, kj * OUT_CH:(kj + 1) * OUT_CH] for kj in range(KJ)]
    A = [nc.alloc_psum_tensor(f"a_ps{i}", [P, 2 * O_CORE], f32).ap()
         for i in range(2)]
    pacc = [nc.alloc_psum_tensor(f"pacc{kj}", [P, 1 + OUT_CH], f32).ap()
            for kj in range(KJ)]

    with (
        nc.Block() as block,
        nc.semaphore("dsem_s") as dsem_s,   # sync-queue input DMAs
        nc.semaphore("dsem_g") as dsem_g,   # gpsimd-queue input DMAs
        nc.semaphore("psem") as psem,       # PE exponent matmuls done
        nc.semaphore("asem") as asem,       # ACT exp pairs done
        nc.semaphore("gsem") as gsem,       # PE agg per-kj done
        nc.semaphore("vsem") as vsem,       # DVE epilogue per-kj done
        nc.semaphore("vv") as vv,           # DVE same-engine pipeline sync
        nc.semaphore("osem") as osem,       # out DMAs done
    ):
        @block.sync
        def _(sync):
            # drain detects DMA completion by polling the ring directly —
            # ~1us faster than the HWDGE completion-semaphore path
            sync.dma_start(out=xr[:], in_=xr_d[:]).then_inc(dsem_s, 16)
            # per-kj output DMAs fire as each epilogue chain lands, so the
            # kj0-2 stores overlap the remaining matmuls/epilogues
            for kj in range(KJ):
                sync.wait_ge(vsem, kj + 1)
                sync.dma_start(
                    out=out_d[kj * P:(kj + 1) * P, :], in_=o_sb[kj][:]
                ).then_inc(osem, 16)
            # no final osem wait: the Block-exit DRAIN on SP already blocks
            # until the DGE queues are empty, and the semaphore completion
            # path adds ~1.8us of latency on top of the actual transfer

        @block.gpsimd
        def _(gpsimd):
            gpsimd.dma_start(out=ydt[:], in_=ydt_d[:]).then_inc(dsem_g, 16)
            gpsimd.dma_start(out=wbb[:], in_=wbb_d[:]).then_inc(dsem_g, 16)

        @block.tensor
        def _(tensor):
            tensor.wait_ge(dsem_s, 16)
            for q in range(npair):
                g, p = divmod(q, NPAIR)
                if q >= 2:
                    tensor.wait_ge(asem, q - 1)  # A buffer q%2 free again
                for half in range(2):
                    ki = 2 * p + half
                    tensor.matmul(
                        A[q % 2][:, half * O_CORE:(half + 1) * O_CORE],
                        lxh[g][:, ki * P:(ki + 1) * P],
                        rth[g][:],
                        start=True,
                        stop=True,
                    ).then_inc(psem, 1)
            tensor.wait_ge(dsem_g, 32)  # ydt (all gpsimd-queue DMAs)
            n_mm = G * KI
            for kj in range(KJ):
                for g in range(G):
                    w = widths[g]
                    coff = 0 if g == 0 else 1
                    for ki in range(KI):
                        idx = g * KI + ki
                        q = g * NPAIR + ki // 2
                        if kj == 0 and ki % 2 == 0:
                            tensor.wait_ge(asem, q + 1)
                        rhs = ydt[:, KI * offs[g] + ki * w:
                                  KI * offs[g] + (ki + 1) * w]
                        lhs = E[q][:, (ki % 2) * O_CORE + kj * P:
                                   (ki % 2) * O_CORE + (kj + 1) * P]
                        mm = tensor.matmul(
                            pacc[kj][:, coff:coff + w],
                            lhs,
                            rhs,
                            start=(idx == 0),
                            stop=(idx == n_mm - 1),
                            skip_group_check=(G > 1),
                        )
                        if idx == n_mm - 1:
                            mm.then_inc(gsem, 1)

        @block.scalar
        def _(scalar):
            # touch Exp before the pipeline needs it: loads the ACT table
            # while the input DMAs are still in flight
            scalar.activation(warm[:], nc.const_aps.tensor(0.0, (1, 1)), Exp)
            for q in range(npair):
                scalar.wait_ge(psem, 2 * (q + 1))
                scalar.activation(E[q][:], A[q % 2][:], Exp).then_inc(asem, 1)

        @block.vector
        def _(vector):
            vector.wait_ge(dsem_g, 32)  # wbb resident
            # per-kj chain: runs as soon as that kj's PSUM bank is complete,
            # so kj0-2 finish during the remaining aggregation matmuls
            for kj in range(KJ):
                vector.wait_ge(gsem, kj + 1)
                vector.tensor_scalar_add(
                    denp[kj][:], pacc[kj][:, 0:1], EPS
                ).then_inc(vv, 1)
                vector.wait_ge(vv, 3 * kj + 1)  # denp through the pipe
                vector.scalar_tensor_tensor(
                    t1[kj][:], wb0[:], denp[kj][:], bt[:],
                    op0=mybir.AluOpType.mult, op1=mybir.AluOpType.add,
                ).then_inc(vv, 1)
                vector.reciprocal(recip[kj][:], denp[kj][:]).then_inc(vv, 1)
                vector.wait_ge(vv, 3 * kj + 3)  # t1 + recip through the pipe
                vector.scalar_tensor_tensor(
                    o_sb[kj][:], pacc[kj][:, 1:1 + OUT_CH], recip[kj][:], t1[kj][:],
                    op0=mybir.AluOpType.mult, op1=mybir.AluOpType.add,
                ).then_inc(vsem, 1)

    return nc


def _build_fp32(widths):
    """Fallback: fp32 rank-3 exponent matmul + fp32 aggregation (slower,
    used only when fp16 split values would overflow)."""
    import concourse.bacc as bacc
    import concourse.tile as tile
    from concourse import mybir

    f32 = mybir.dt.float32
    G = len(widths)
    wtot = sum(widths)
    offs = np.cumsum([0] + list(widths))

    nc = bacc.Bacc("TRN2", target_bir_lowering=False, debug=False)

    lx_d = nc.dram_tensor("lx", [3, N_IN], f32, kind="ExternalInput")
    rt_d = nc.dram_tensor("rt", [3 * G, O_CORE], f32, kind="ExternalInput")
    ydt_d = nc.dram_tensor("ydt", [P, KI * wtot], f32, kind="ExternalInput")
    wb0_d = nc.dram_tensor("wb0", [P, OUT_CH], f32, kind="ExternalInput")
    bt_d = nc.dram_tensor("bt", [P, OUT_CH], f32, kind="ExternalInput")
    out_d = nc.dram_tensor("out", [O_CORE, OUT_CH], f32, kind="ExternalOutput")

    with tile.TileContext(nc) as tc:
        with (
            tc.tile_pool(name="const", bufs=1) as cpool,
            tc.tile_pool(name="epool", bufs=1) as epool,
            tc.tile_pool(name="small", bufs=2) as spool,
            tc.tile_pool(name="outp", bufs=2) as opool,
            tc.tile_pool(name="apsum", bufs=3, space="PSUM") as apsum,
            tc.tile_pool(name="ppsum", bufs=1, space="PSUM") as ppsum,
        ):
            lx = cpool.tile([3, N_IN], f32, tag="lx")
            nc.sync.dma_start(lx[:], lx_d[:])
            rt = cpool.tile([3 * G, O_CORE], f32, tag="rt")
            nc.scalar.dma_start(rt[:], rt_d[:])
            ydt = cpool.tile([P, KI * wtot], f32, tag="ydt")
            nc.gpsimd.dma_start(ydt[:], ydt_d[:])
            wb0 = cpool.tile([P, OUT_CH], f32, tag="wb0")
            nc.gpsimd.dma_start(wb0[:], wb0_d[:])
            bt = cpool.tile([P, OUT_CH], f32, tag="bt")
            nc.gpsimd.dma_start(bt[:], bt_d[:])

            E = {}
            for g in range(G):
                for ki in range(KI):
                    a_ps = apsum.tile([P, O_CORE], f32, tag="A", name="a_ps")
                    nc.tensor.matmul(
                        a_ps[:],
                        lx[:, ki * P:(ki + 1) * P],
                        rt[3 * g:3 * g + 3, :],
                        start=True,
                        stop=True,
                    )
                    e = epool.tile([P, O_CORE], f32, tag=f"E{g}_{ki}", name="e")
                    nc.scalar.activation(
                        e[:], a_ps[:], mybir.ActivationFunctionType.Exp
                    )
                    E[(g, ki)] = e

            pacc = [
                ppsum.tile([P, 1 + OUT_CH], f32, tag=f"P{kj}", name=f"pacc{kj}")
                for kj in range(KJ)
            ]
            n_mm = G * KI
            for kj in range(KJ):
                for g in range(G):
                    w = widths[g]
                    coff = 0 if g == 0 else 1
                    for ki in range(KI):
                        idx = g * KI + ki
                        rhs = ydt[:, KI * offs[g] + ki * w: KI * offs[g] + (ki + 1) * w]
                        nc.tensor.matmul(
                            pacc[kj][:, coff:coff + w],
                            E[(g, ki)][:, kj * P:(kj + 1) * P],
                            rhs,
                            start=(idx == 0),
                            stop=(idx == n_mm - 1),
                            skip_group_check=(G > 1),
                        )

                denp = spool.tile([P, 1], f32, tag="denp", name="denp")
                nc.vector.tensor_scalar_add(denp[:], pacc[kj][:, 0:1], EPS)
                recip = spool.tile([P, 1], f32, tag="recip", name="recip")
                nc.vector.reciprocal(recip[:], denp[:])
                t1 = spool.tile([P, OUT_CH], f32, tag="t1", name="t1")
                nc.vector.scalar_tensor_tensor(
                    t1[:], wb0[:], denp[:], bt[:],
                    op0=mybir.AluOpType.mult, op1=mybir.AluOpType.add,
                )
                o_sb = opool.tile([P, OUT_CH], f32, tag="osb", name="o_sb")
                nc.vector.scalar_tensor_tensor(
                    o_sb[:], pacc[kj][:, 1:1 + OUT_CH], recip[:], t1[:],
                    op0=mybir.AluOpType.mult, op1=mybir.AluOpType.add,
                )
                nc.sync.dma_start(out_d[kj * P:(kj + 1) * P, :], o_sb[:])

    nc.compile()
    return nc


def _split2_f16(v):
    """2-way fp16 split: v ~= h1 + h2 with each half exactly fp16."""
    v = v.astype(np.float32)
    h1 = v.astype(np.float16)
    h2 = (v - h1.astype(np.float32)).astype(np.float16)
    return h1, h2


def _prepare_inputs(context_x, context_y, t, sigma, W, b):
    """Host prep: group channels by sigma, fold W, build per-core inputs."""
    sigma = np.asarray(sigma, dtype=np.float32)
    W64 = np.asarray(W, dtype=np.float64)
    b64 = np.asarray(b, dtype=np.float64)

    uniq = []
    for c in range(IN_CH):
        if sigma[c] not in uniq:
            uniq.append(sigma[c])
    uniq.sort(key=lambda s: (s != sigma[0]))  # channel-0 group first
    groups = [[c for c in range(IN_CH) if sigma[c] == s] for s in uniq]
    alphas = [0.5 / np.exp(2.0 * np.float64(s)) for s in uniq]
    widths = tuple((1 + OUT_CH) if 0 in g else OUT_CH for g in groups)
    G = len(groups)

    # fp16 path is safe unless sqrt(2a)*x or a*x^2 style terms overflow.
    xmax = max(
        float(np.abs(np.asarray(context_x)).max()),
        float(np.abs(np.asarray(t)).max()),
        1.0,
    )
    fp16_ok = all(a * xmax * xmax < 3e4 and np.isfinite(a) for a in alphas)

    in_maps = []
    for core in range(N_CORES):
        bidx, half = core // 2, core % 2
        x = np.asarray(context_x[bidx, :, 0], dtype=np.float64)
        th = np.asarray(t[bidx, half * O_CORE:(half + 1) * O_CORE, 0],
                        dtype=np.float64)
        y = np.asarray(context_y[bidx], dtype=np.float64)

        m = {}
        if fp16_ok:
            BLK = N_IN + O_CORE
            xr = np.empty((KEXP, G * BLK), dtype=np.float16)
            for g, a in enumerate(alphas):
                r = np.sqrt(2.0 * a)
                s1, s2 = _split2_f16(r * x)
                u1, u2 = _split2_f16(r * th)
                q1, q2 = _split2_f16(0.5 * (r * x) ** 2)
                w1, w2 = _split2_f16(0.5 * (r * th) ** 2)
                one_i = np.ones(N_IN, np.float16)
                neg1 = np.full(O_CORE, -1.0, np.float16)
                xr[:, g * BLK:g * BLK + N_IN] = np.stack(
                    [s1, s1, s2, s2, q1, q2, one_i, one_i]
                )
                xr[:, g * BLK + N_IN:(g + 1) * BLK] = np.stack(
                    [u1, u2, u1, u2, neg1, neg1, -w1, -w2]
                )
            m["xr"] = xr
        else:
            lx = np.stack([x, x * x, np.ones_like(x)]).astype(np.float32)
            rt = np.empty((3 * G, O_CORE), dtype=np.float32)
            for g, a in enumerate(alphas):
                rt[3 * g + 0] = 2.0 * a * th
                rt[3 * g + 1] = -a
                rt[3 * g + 2] = -a * th * th
            m["lx"], m["rt"] = lx, rt

        blocks = []
        for g, chans in enumerate(groups):
            w = widths[g]
            rhs = np.zeros((N_IN, w), dtype=np.float64)
            coff = 0
            if 0 in chans:
                rhs[:, 0] = 1.0
                coff = 1
            conv_ch = [c for c in chans if c > 0]
            if conv_ch:
                rhs[:, coff:] = y[:, [c - 1 for c in conv_ch]] @ W64[conv_ch, :]
            blocks.append(
                rhs.reshape(KI, P, w).transpose(1, 0, 2).reshape(P, KI * w)
            )
        ydt = np.concatenate(blocks, axis=1)
        m["ydt"] = ydt.astype(np.float16 if fp16_ok else np.float32)
        wb0 = np.tile(W64[0].astype(np.float32), (P, 1))
        bt = np.tile(b64.astype(np.float32), (P, 1))
        if fp16_ok:
            m["wbb"] = np.concatenate([wb0, bt], axis=1)
        else:
            m["wb0"], m["bt"] = wb0, bt
        in_maps.append(m)
    return widths, fp16_ok, in_maps


def _run(inputs: dict, trace: bool = False):
    """Compile (cached), run on 8 cores, gather. Returns (output, results)."""
    from concourse.bass_utils import run_bass_kernel_spmd

    widths, fp16_ok, in_maps = _prepare_inputs(
        inputs["context_x"], inputs["context_y"], inputs["t"],
        inputs["sigma"], inputs["W"], inputs["b"],
    )
    key = (widths, fp16_ok)
    if key not in _BASS_CACHE:
        _BASS_CACHE[key] = (_build_fp16_raw if fp16_ok else _build_fp32)(widths)
    nc = _BASS_CACHE[key]

    res = run_bass_kernel_spmd(nc, in_maps, list(range(N_CORES)), trace=trace)

    out = np.empty((B, N_OUT, OUT_CH), dtype=np.float32)
    for core in range(N_CORES):
        bidx, half = core // 2, core % 2
        out[bidx, half * O_CORE:(half + 1) * O_CORE, :] = res.results[core]["out"]
    return out, res


def kernel(**inputs) -> np.ndarray:
    out, _ = _run(inputs, trace=False)
    return out



# revision 8
# speedup vs baseline: 1.1549x; 1.1549x over previous
"""ConvDeepSet kernel for Trainium2 (8 NeuronCores, Bass/Tile).

Math (per batch b, target point o, channel c):
    agg[o,c] = sum_i yd[i,c] * exp(-alpha_c * (x_i - t_o)^2)      yd = [1 | y]
    out[o,:] = [agg0, agg1/(agg0+eps), ...] @ W + b

Fast path (banded, all sigmas equal): targets are sorted per batch on the
host; each 128-target block only interacts with a 512-wide window of the
sorted context points (RBF length scale 0.1 => weights beyond the window
are < 1e-8; the host validates the truncation error exactly in float64 and
falls back to the dense kernel if it exceeds tolerance).

Per core (batch b, half h of the sorted targets), per block k (128 targets):
  - exponent: -a(x-t)^2 = s*u - s^2/2 - u^2/2 via a rank-8 fp16 matmul of
    host-prepared 2-way fp16 splits (exact to ~1e-4).  The 4 window chunks
    are packed in the PE array with row-group tiling (contract dim 8 -> 4
    concurrent matmuls at tile_position (32c, 0)).
  - exp on ScalarE per block: PSUM [128,512] -> SBUF fp16.
  - aggregation: 4 matmuls per block, E chunk stationary, ydt (W folded in
    on the host) moving; all blocks accumulate into ONE PSUM bank
    pacc[128, 4*17] so the epilogue is 4 wide DVE ops instead of 16:
      recip = 1/den (strided), t1 = W0*den (broadcast APs),
      tmp = conv*recip, out = tmp + t1.   (eps dropped: den >> eps,
    validated on the host.)
  - single merged input DMA per queue and a single [128, 64] output DMA;
    the host un-sorts the targets when gathering.

Dense fp16 and fp32 variants are kept as fallbacks for inputs where the
banded assumptions (single sigma group, fp16-safe ranges, small truncation
error) do not hold.
"""

import numpy as np

B, N_IN, N_OUT = 4, 1024, 1024
IN_CH_RAW, OUT_CH = 7, 16
IN_CH = IN_CH_RAW + 1
N_CORES = 8
O_CORE = N_OUT // 2          # 512 target points per core
P = 128                      # partitions
KI = N_IN // P               # 8 contraction chunks (dense path)
NPAIR = KI // 2              # exp processed in chunk pairs (dense path)
KJ = O_CORE // P             # 4 output chunks
KEXP = 8                     # rows of the split-product exponent matmul
EPS = 1e-8

# banded-path geometry
WCTX = 512                   # context window per 128-target block
CH = WCTX // P               # 4 window chunks
BLKS = KJ                    # 4 target blocks per core
XRB = WCTX + P               # xr2 cols per block: 512 lhs + 128 rhs

_BASS_CACHE: dict = {}


def _build_banded(has_bias: bool):
    """Banded fp16 kernel: sorted targets, 512-wide context windows."""
    import concourse.bass as bass
    from concourse import mybir

    f32 = mybir.dt.float32
    f16 = mybir.dt.float16
    u8 = mybir.dt.uint8
    Exp = mybir.ActivationFunctionType.Exp
    Mult = mybir.AluOpType.mult

    # byte layout of the merged gpsimd DMA: w4 f32 | (bt f32) | ydt f16
    W4B = KJ * OUT_CH * 4                 # 256
    BTB = KJ * OUT_CH * 4 if has_bias else 0
    YDTB = BLKS * CH * 17 * 2             # 544
    NB = W4B + BTB + YDTB

    nc = bass.Bass("TRN2", target_bir_lowering=False, debug=False)

    xr2_d = nc.dram_tensor("xr2", [KEXP, BLKS * XRB], f16, kind="ExternalInput")
    ydtw_d = nc.dram_tensor("ydtw", [P, NB], u8, kind="ExternalInput")
    out_d = nc.dram_tensor("out", [P, KJ * OUT_CH], f32, kind="ExternalOutput")

    xr2 = nc.alloc_sbuf_tensor("xr2_sb", [KEXP, BLKS * XRB], f16).ap()
    big = nc.alloc_sbuf_tensor("big_sb", [P, NB], u8).ap()
    w4 = big[:, 0:W4B].bitcast(f32)                       # [128, 64]
    bt = big[:, W4B:W4B + BTB].bitcast(f32) if has_bias else None
    ydt = big[:, W4B + BTB:NB].bitcast(f16)               # [128, 272]
    warm_w = nc.alloc_sbuf_tensor("warm_w_sb", [KEXP, 512], f16).ap()
    warm = nc.alloc_sbuf_tensor("warm_sb", [1, 1], f32).ap()
    E = [nc.alloc_sbuf_tensor(f"e_sb{k}", [P, WCTX], f16).ap()
         for k in range(BLKS)]
    rec = nc.alloc_sbuf_tensor("rec_sb", [P, KJ], f32).ap()
    t1 = nc.alloc_sbuf_tensor("t1_sb", [P, KJ * OUT_CH], f32).ap()
    tmp = nc.alloc_sbuf_tensor("tmp_sb", [P, KJ * OUT_CH], f32).ap()
    o_all = nc.alloc_sbuf_tensor("o_all_sb", [P, KJ * OUT_CH], f32).ap()

    A = [nc.alloc_psum_tensor(f"a_ps{i}", [P, WCTX], f32).ap()
         for i in range(2)]
    pacc = nc.alloc_psum_tensor("pacc_ps", [P, KJ * 17], f32).ap()
    warm_ps = nc.alloc_psum_tensor("warm_ps", [KEXP, 512], f32).ap()

    pacc3 = pacc.rearrange("p (k c) -> p k c", k=KJ)      # [128, 4, 17]
    den = pacc3[:, :, 0]                                  # [128, 4] strided
    conv3 = pacc3[:, :, 1:17]                             # [128, 4, 16]
    den_b = den.unsqueeze(2).broadcast_to([P, KJ, OUT_CH])
    rec_b = rec.unsqueeze(2).broadcast_to([P, KJ, OUT_CH])
    w4_3 = w4.rearrange("p (k c) -> p k c", k=KJ)
    t1_3 = t1.rearrange("p (k c) -> p k c", k=KJ)
    tmp_3 = tmp.rearrange("p (k c) -> p k c", k=KJ)

    with (
        nc.Block() as block,
        nc.semaphore("dsem_s") as dsem_s,   # sync-queue input DMA (xr2)
        nc.semaphore("dsem_g") as dsem_g,   # gpsimd-queue input DMA (ydtw)
        nc.semaphore("psem") as psem,       # PE exponent blocks done
        nc.semaphore("asem") as asem,       # ACT exp blocks done
        nc.semaphore("gsem") as gsem,       # PE aggregation done
        nc.semaphore("vv") as vv,           # DVE same-engine pipeline sync
        nc.semaphore("vsem") as vsem,       # DVE epilogue done
        nc.semaphore("osem") as osem,       # out DMA done (never waited on;
                                            # the Block-exit DRAIN on SP
                                            # blocks until DGE queues empty)
    ):
        @block.sync
        def _(sync):
            sync.dma_start(out=xr2[:], in_=xr2_d[:]).then_inc(dsem_s, 16)
            sync.wait_ge(vsem, 1)
            sync.dma_start(out=out_d[:], in_=o_all[:]).then_inc(osem, 16)

        @block.gpsimd
        def _(gpsimd):
            gpsimd.dma_start(out=big[:], in_=ydtw_d[:]).then_inc(dsem_g, 16)

        @block.tensor
        def _(tensor):
            # HAM warmup while the input DMAs are in flight (uninitialized
            # operands; results discarded)
            for _w in range(3):
                tensor.matmul(warm_ps[:], warm_w[:, 0:KEXP], warm_w[:],
                              start=True, stop=True)
            tensor.wait_ge(dsem_s, 16)
            for k in range(BLKS):
                if k >= 2:
                    tensor.wait_ge(asem, k - 1)   # A[k%2] free again
                base = k * XRB
                for c in range(CH):
                    mm = tensor.matmul(
                        A[k % 2][:, c * P:(c + 1) * P],
                        xr2[:, base + c * P:base + (c + 1) * P],
                        xr2[:, base + WCTX:base + XRB],
                        start=True,
                        stop=True,
                    )
                    if c == CH - 1:
                        mm.then_inc(psem, 1)
            tensor.wait_ge(dsem_g, 16)
            for k in range(BLKS):
                tensor.wait_ge(asem, k + 1)
                for c in range(CH):
                    mm = tensor.matmul(
                        pacc[:, k * 17:(k + 1) * 17],
                        E[k][:, c * P:(c + 1) * P],
                        ydt[:, 17 * (CH * k + c):17 * (CH * k + c + 1)],
                        start=(c == 0),
                        stop=(c == CH - 1),
                        skip_group_check=True,
                    )
                    if k == BLKS - 1 and c == CH - 1:
                        mm.then_inc(gsem, 1)

        @block.scalar
        def _(scalar):
            # touch Exp before the pipeline needs it: loads the ACT table
            # while the input DMAs are still in flight
            scalar.activation(warm[:], nc.const_aps.tensor(0.0, (1, 1)), Exp)
            for k in range(BLKS):
                scalar.wait_ge(psem, k + 1)
                scalar.activation(E[k][:], A[k % 2][:], Exp).then_inc(asem, 1)

        @block.vector
        def _(vector):
            vector.wait_ge(dsem_g, 16)   # w4 resident
            vector.wait_ge(gsem, 1)      # pacc complete
            vector.reciprocal(rec[:], den).then_inc(vv, 1)
            vector.tensor_tensor(t1_3, w4_3, den_b, Mult).then_inc(vv, 1)
            vector.wait_ge(vv, 2)
            vector.tensor_tensor(tmp_3, conv3, rec_b, Mult).then_inc(vv, 1)
            vector.wait_ge(vv, 3)
            if has_bias:
                vector.scalar_tensor_tensor(
                    o_all[:], tmp[:], 1.0, t1[:],
                    op0=Mult, op1=mybir.AluOpType.add,
                ).then_inc(vv, 1)
                vector.wait_ge(vv, 4)
                vector.tensor_add(o_all[:], o_all[:], bt[:]).then_inc(vsem, 1)
            else:
                vector.tensor_add(o_all[:], tmp[:], t1[:]).then_inc(vsem, 1)

    return nc


def _build_fp16_raw(widths):
    """Dense fallback: raw-Bass fp16 pipeline (no banding)."""
    import concourse.bass as bass
    from concourse import mybir

    f32 = mybir.dt.float32
    f16 = mybir.dt.float16
    G = len(widths)
    wtot = sum(widths)
    offs = np.cumsum([0] + list(widths))
    npair = NPAIR * G
    Exp = mybir.ActivationFunctionType.Exp

    nc = bass.Bass("TRN2", target_bir_lowering=False, debug=False)

    BLK = N_IN + O_CORE
    xr_d = nc.dram_tensor("xr", [KEXP, G * BLK], f16, kind="ExternalInput")
    ydt_d = nc.dram_tensor("ydt", [P, KI * wtot], f16, kind="ExternalInput")
    wbb_d = nc.dram_tensor("wbb", [P, 2 * OUT_CH], f32, kind="ExternalInput")
    out_d = nc.dram_tensor("out", [O_CORE, OUT_CH], f32, kind="ExternalOutput")

    xr = nc.alloc_sbuf_tensor("xr_sb", [KEXP, G * BLK], f16).ap()
    lxh = [xr[:, g * BLK:g * BLK + N_IN] for g in range(G)]
    rth = [xr[:, g * BLK + N_IN:(g + 1) * BLK] for g in range(G)]
    ydt = nc.alloc_sbuf_tensor("ydt_sb", [P, KI * wtot], f16).ap()
    wbb = nc.alloc_sbuf_tensor("wbb_sb", [P, 2 * OUT_CH], f32).ap()
    wb0 = wbb[:, :OUT_CH]
    bt = wbb[:, OUT_CH:]
    warm = nc.alloc_sbuf_tensor("warm_sb", [1, 1], f32).ap()
    E = [nc.alloc_sbuf_tensor(f"e_sb{q}", [P, 2 * O_CORE], f16).ap()
         for q in range(npair)]
    denp = [nc.alloc_sbuf_tensor(f"denp_sb{kj}", [P, 1], f32).ap()
            for kj in range(KJ)]
    recip = [nc.alloc_sbuf_tensor(f"recip_sb{kj}", [P, 1], f32).ap()
             for kj in range(KJ)]
    t1 = [nc.alloc_sbuf_tensor(f"t1_sb{kj}", [P, OUT_CH], f32).ap()
          for kj in range(KJ)]
    o_all = nc.alloc_sbuf_tensor("o_all_sb", [P, KJ * OUT_CH], f32).ap()
    o_sb = [o_all[:, kj * OUT_CH:(kj + 1) * OUT_CH] for kj in range(KJ)]
    A = [nc.alloc_psum_tensor(f"a_ps{i}", [P, 2 * O_CORE], f32).ap()
         for i in range(2)]
    pacc = [nc.alloc_psum_tensor(f"pacc{kj}", [P, 1 + OUT_CH], f32).ap()
            for kj in range(KJ)]

    with (
        nc.Block() as block,
        nc.semaphore("dsem_s") as dsem_s,
        nc.semaphore("dsem_g") as dsem_g,
        nc.semaphore("psem") as psem,
        nc.semaphore("asem") as asem,
        nc.semaphore("gsem") as gsem,
        nc.semaphore("vsem") as vsem,
        nc.semaphore("vv") as vv,
        nc.semaphore("osem") as osem,
    ):
        @block.sync
        def _(sync):
            sync.dma_start(out=xr[:], in_=xr_d[:]).then_inc(dsem_s, 16)
            for kj in range(KJ):
                sync.wait_ge(vsem, kj + 1)
                sync.dma_start(
                    out=out_d[kj * P:(kj + 1) * P, :], in_=o_sb[kj][:]
                ).then_inc(osem, 16)

        @block.gpsimd
        def _(gpsimd):
            gpsimd.dma_start(out=ydt[:], in_=ydt_d[:]).then_inc(dsem_g, 16)
            gpsimd.dma_start(out=wbb[:], in_=wbb_d[:]).then_inc(dsem_g, 16)

        @block.tensor
        def _(tensor):
            tensor.wait_ge(dsem_s, 16)
            for q in range(npair):
                g, p = divmod(q, NPAIR)
                if q >= 2:
                    tensor.wait_ge(asem, q - 1)
                for half in range(2):
                    ki = 2 * p + half
                    tensor.matmul(
                        A[q % 2][:, half * O_CORE:(half + 1) * O_CORE],
                        lxh[g][:, ki * P:(ki + 1) * P],
                        rth[g][:],
                        start=True,
                        stop=True,
                    ).then_inc(psem, 1)
            tensor.wait_ge(dsem_g, 32)
            n_mm = G * KI
            for kj in range(KJ):
                for g in range(G):
                    w = widths[g]
                    coff = 0 if g == 0 else 1
                    for ki in range(KI):
                        idx = g * KI + ki
                        q = g * NPAIR + ki // 2
                        if kj == 0 and ki % 2 == 0:
                            tensor.wait_ge(asem, q + 1)
                        rhs = ydt[:, KI * offs[g] + ki * w:
                                  KI * offs[g] + (ki + 1) * w]
                        lhs = E[q][:, (ki % 2) * O_CORE + kj * P:
                                   (ki % 2) * O_CORE + (kj + 1) * P]
                        mm = tensor.matmul(
                            pacc[kj][:, coff:coff + w],
                            lhs,
                            rhs,
                            start=(idx == 0),
                            stop=(idx == n_mm - 1),
                            skip_group_check=(G > 1),
                        )
                        if idx == n_mm - 1:
                            mm.then_inc(gsem, 1)

        @block.scalar
        def _(scalar):
            scalar.activation(warm[:], nc.const_aps.tensor(0.0, (1, 1)), Exp)
            for q in range(npair):
                scalar.wait_ge(psem, 2 * (q + 1))
                scalar.activation(E[q][:], A[q % 2][:], Exp).then_inc(asem, 1)

        @block.vector
        def _(vector):
            vector.wait_ge(dsem_g, 32)
            for kj in range(KJ):
                vector.wait_ge(gsem, kj + 1)
                vector.tensor_scalar_add(
                    denp[kj][:], pacc[kj][:, 0:1], EPS
                ).then_inc(vv, 1)
                vector.wait_ge(vv, 3 * kj + 1)
                vector.scalar_tensor_tensor(
                    t1[kj][:], wb0[:], denp[kj][:], bt[:],
                    op0=mybir.AluOpType.mult, op1=mybir.AluOpType.add,
                ).then_inc(vv, 1)
                vector.reciprocal(recip[kj][:], denp[kj][:]).then_inc(vv, 1)
                vector.wait_ge(vv, 3 * kj + 3)
                vector.scalar_tensor_tensor(
                    o_sb[kj][:], pacc[kj][:, 1:1 + OUT_CH], recip[kj][:], t1[kj][:],
                    op0=mybir.AluOpType.mult, op1=mybir.AluOpType.add,
                ).then_inc(vsem, 1)

    return nc


def _build_fp32(widths):
    """Fallback: fp32 rank-3 exponent matmul + fp32 aggregation (slower,
    used only when fp16 split values would overflow)."""
    import concourse.bacc as bacc
    import concourse.tile as tile
    from concourse import mybir

    f32 = mybir.dt.float32
    G = len(widths)
    wtot = sum(widths)
    offs = np.cumsum([0] + list(widths))

    nc = bacc.Bacc("TRN2", target_bir_lowering=False, debug=False)

    lx_d = nc.dram_tensor("lx", [3, N_IN], f32, kind="ExternalInput")
    rt_d = nc.dram_tensor("rt", [3 * G, O_CORE], f32, kind="ExternalInput")
    ydt_d = nc.dram_tensor("ydt", [P, KI * wtot], f32, kind="ExternalInput")
    wb0_d = nc.dram_tensor("wb0", [P, OUT_CH], f32, kind="ExternalInput")
    bt_d = nc.dram_tensor("bt", [P, OUT_CH], f32, kind="ExternalInput")
    out_d = nc.dram_tensor("out", [O_CORE, OUT_CH], f32, kind="ExternalOutput")

    with tile.TileContext(nc) as tc:
        with (
            tc.tile_pool(name="const", bufs=1) as cpool,
            tc.tile_pool(name="epool", bufs=1) as epool,
            tc.tile_pool(name="small", bufs=2) as spool,
            tc.tile_pool(name="outp", bufs=2) as opool,
            tc.tile_pool(name="apsum", bufs=3, space="PSUM") as apsum,
            tc.tile_pool(name="ppsum", bufs=1, space="PSUM") as ppsum,
        ):
            lx = cpool.tile([3, N_IN], f32, tag="lx")
            nc.sync.dma_start(lx[:], lx_d[:])
            rt = cpool.tile([3 * G, O_CORE], f32, tag="rt")
            nc.scalar.dma_start(rt[:], rt_d[:])
            ydt = cpool.tile([P, KI * wtot], f32, tag="ydt")
            nc.gpsimd.dma_start(ydt[:], ydt_d[:])
            wb0 = cpool.tile([P, OUT_CH], f32, tag="wb0")
            nc.gpsimd.dma_start(wb0[:], wb0_d[:])
            bt = cpool.tile([P, OUT_CH], f32, tag="bt")
            nc.gpsimd.dma_start(bt[:], bt_d[:])

            E = {}
            for g in range(G):
                for ki in range(KI):
                    a_ps = apsum.tile([P, O_CORE], f32, tag="A", name="a_ps")
                    nc.tensor.matmul(
                        a_ps[:],
                        lx[:, ki * P:(ki + 1) * P],
                        rt[3 * g:3 * g + 3, :],
                        start=True,
                        stop=True,
                    )
                    e = epool.tile([P, O_CORE], f32, tag=f"E{g}_{ki}", name="e")
                    nc.scalar.activation(
                        e[:], a_ps[:], mybir.ActivationFunctionType.Exp
                    )
                    E[(g, ki)] = e

            pacc = [
                ppsum.tile([P, 1 + OUT_CH], f32, tag=f"P{kj}", name=f"pacc{kj}")
                for kj in range(KJ)
            ]
            n_mm = G * KI
            for kj in range(KJ):
                for g in range(G):
                    w = widths[g]
                    coff = 0 if g == 0 else 1
                    for ki in range(KI):
                        idx = g * KI + ki
                        rhs = ydt[:, KI * offs[g] + ki * w: KI * offs[g] + (ki + 1) * w]
                        nc.tensor.matmul(
                            pacc[kj][:, coff:coff + w],
                            E[(g, ki)][:, kj * P:(kj + 1) * P],
                            rhs,
                            start=(idx == 0),
                            stop=(idx == n_mm - 1),
                            skip_group_check=(G > 1),
                        )

                denp = spool.tile([P, 1], f32, tag="denp", name="denp")
                nc.vector.tensor_scalar_add(denp[:], pacc[kj][:, 0:1], EPS)
                recip = spool.tile([P, 1], f32, tag="recip", name="recip")
                nc.vector.reciprocal(recip[:], denp[:])
                t1 = spool.tile([P, OUT_CH], f32, tag="t1", name="t1")
                nc.vector.scalar_tensor_tensor(
                    t1[:], wb0[:], denp[:], bt[:],
                    op0=mybir.AluOpType.mult, op1=mybir.AluOpType.add,
                )
                o_sb = opool.tile([P, OUT_CH], f32, tag="osb", name="o_sb")
                nc.vector.scalar_tensor_tensor(
                    o_sb[:], pacc[kj][:, 1:1 + OUT_CH], recip[:], t1[:],
                    op0=mybir.AluOpType.mult, op1=mybir.AluOpType.add,
                )
                nc.sync.dma_start(out_d[kj * P:(kj + 1) * P, :], o_sb[:])

    nc.compile()
    return nc


def _split2_f16(v):
    """2-way fp16 split: v ~= h1 + h2 with each half exactly fp16."""
    v = v.astype(np.float32)
    h1 = v.astype(np.float16)
    h2 = (v - h1.astype(np.float32)).astype(np.float16)
    return h1, h2


def _sigma_groups(sigma):
    sigma = np.asarray(sigma, dtype=np.float32)
    uniq = []
    for c in range(IN_CH):
        if sigma[c] not in uniq:
            uniq.append(sigma[c])
    uniq.sort(key=lambda s: (s != sigma[0]))  # channel-0 group first
    groups = [[c for c in range(IN_CH) if sigma[c] == s] for s in uniq]
    alphas = [0.5 / np.exp(2.0 * np.float64(s)) for s in uniq]
    widths = tuple((1 + OUT_CH) if 0 in g else OUT_CH for g in groups)
    return groups, alphas, widths


def _try_prepare_banded(context_x, context_y, t, sigma, W, b):
    """Banded host prep.  Returns (in_maps, scatter, has_bias) or None if
    the banded assumptions fail on this input (multi-sigma, fp16-unsafe
    ranges, or truncation error above tolerance -- all checked exactly)."""
    groups, alphas, widths = _sigma_groups(sigma)
    if len(groups) != 1:
        return None
    a = float(alphas[0])
    if not np.isfinite(a):
        return None
    xmax = max(
        float(np.abs(np.asarray(context_x)).max()),
        float(np.abs(np.asarray(t)).max()),
        1.0,
    )
    if not (a * xmax * xmax < 3e4):
        return None

    W64 = np.asarray(W, dtype=np.float64)
    b64 = np.asarray(b, dtype=np.float64)
    has_bias = bool(np.any(b64 != 0.0))

    x_all = np.asarray(context_x, np.float64)[:, :, 0]
    t_all = np.asarray(t, np.float64)[:, :, 0]
    y_all = np.asarray(context_y, np.float64)

    # validate truncation exactly (float64) and gather the windows
    in_maps = [None] * N_CORES
    scatter = [None] * N_CORES
    err_max = 0.0
    den_min = np.inf
    for bidx in range(B):
        x = x_all[bidx]
        xs_idx = np.argsort(x, kind="stable")
        xs = x[xs_idx]
        ts_idx = np.argsort(t_all[bidx], kind="stable")
        ts = t_all[bidx][ts_idx]
        yd = np.empty((N_IN, 1 + OUT_CH))
        yd[:, 0] = 1.0
        yd[:, 1:] = y_all[bidx] @ W64[1:, :]

        # exact reference aggregation per block + banded version
        for half in range(2):
            core = bidx * 2 + half
            xr2 = np.empty((KEXP, BLKS * XRB), dtype=np.float16)
            ydtb = np.empty((P, BLKS * CH * 17), dtype=np.float16)
            for k in range(BLKS):
                pos = half * O_CORE + k * P
                tb = ts[pos:pos + P]
                mid = 0.5 * (tb[0] + tb[-1])
                cpos = np.searchsorted(xs, mid)
                lo = int(np.clip(cpos - WCTX // 2, 0, N_IN - WCTX))
                w_idx = xs_idx[lo:lo + WCTX]
                xw = x[w_idx]

                # truncation error (exact, float64)
                excl = np.concatenate([xs_idx[:lo], xs_idx[lo + WCTX:]])
                Ee = np.exp(-a * (x[excl][:, None] - tb[None, :]) ** 2)
                d_agg = Ee.T @ yd[excl]                  # (128, 17)
                Ew = np.exp(-a * (xw[:, None] - tb[None, :]) ** 2)
                den_w = Ew.T @ yd[w_idx, 0]              # (128,)
                den_min = min(den_min, float(den_w.min()))
                # |d out| <= |d den|*|W0| + (|d conv| + |conv/den|*|d den|)/den
                conv_w = Ew.T @ yd[w_idx, 1:]
                ratio = np.abs(conv_w) / den_w[:, None]
                e_out = (np.abs(d_agg[:, 0:1]) * np.abs(W64[0]) +
                         (np.abs(d_agg[:, 1:]) + ratio * d_agg[:, 0:1]) / den_w[:, None])
                err_max = max(err_max, float(e_out.max()))

                # device data
                r = np.sqrt(2.0 * a)
                s1, s2 = _split2_f16(r * xw)
                q1, q2 = _split2_f16(0.5 * (r * xw) ** 2)
                u1, u2 = _split2_f16(r * tb)
                v1, v2 = _split2_f16(0.5 * (r * tb) ** 2)
                one_i = np.ones(WCTX, np.float16)
                neg1 = np.full(P, -1.0, np.float16)
                base = k * XRB
                xr2[:, base:base + WCTX] = np.stack(
                    [s1, s1, s2, s2, q1, q2, one_i, one_i])
                xr2[:, base + WCTX:base + XRB] = np.stack(
                    [u1, u2, u1, u2, neg1, neg1, -v1, -v2])
                for c in range(CH):
                    sl = slice(c * P, (c + 1) * P)
                    ydtb[:, 17 * (CH * k + c):17 * (CH * k + c + 1)] = \
                        yd[w_idx[sl]].astype(np.float16)

            w4 = np.tile(W64[0].astype(np.float32), (P, KJ))
            parts = [w4.view(np.uint8).reshape(P, -1)]
            if has_bias:
                btile = np.tile(b64.astype(np.float32), (P, KJ))
                parts.append(btile.view(np.uint8).reshape(P, -1))
            parts.append(ydtb.view(np.uint8).reshape(P, -1))
            ydtw = np.concatenate(parts, axis=1)

            in_maps[core] = {"xr2": xr2, "ydtw": ydtw}
            scatter[core] = ts_idx[half * O_CORE:(half + 1) * O_CORE]

    if err_max > 2.5e-3 or den_min < 1e-6:
        return None
    return in_maps, scatter, has_bias


def _prepare_inputs(context_x, context_y, t, sigma, W, b):
    """Dense host prep: group channels by sigma, fold W, per-core inputs."""
    sigma = np.asarray(sigma, dtype=np.float32)
    W64 = np.asarray(W, dtype=np.float64)
    b64 = np.asarray(b, dtype=np.float64)

    groups, alphas, widths = _sigma_groups(sigma)
    G = len(groups)

    xmax = max(
        float(np.abs(np.asarray(context_x)).max()),
        float(np.abs(np.asarray(t)).max()),
        1.0,
    )
    fp16_ok = all(a * xmax * xmax < 3e4 and np.isfinite(a) for a in alphas)

    in_maps = []
    for core in range(N_CORES):
        bidx, half = core // 2, core % 2
        x = np.asarray(context_x[bidx, :, 0], dtype=np.float64)
        th = np.asarray(t[bidx, half * O_CORE:(half + 1) * O_CORE, 0],
                        dtype=np.float64)
        y = np.asarray(context_y[bidx], dtype=np.float64)

        m = {}
        if fp16_ok:
            BLK = N_IN + O_CORE
            xr = np.empty((KEXP, G * BLK), dtype=np.float16)
            for g, a in enumerate(alphas):
                r = np.sqrt(2.0 * a)
                s1, s2 = _split2_f16(r * x)
                u1, u2 = _split2_f16(r * th)
                q1, q2 = _split2_f16(0.5 * (r * x) ** 2)
                w1, w2 = _split2_f16(0.5 * (r * th) ** 2)
                one_i = np.ones(N_IN, np.float16)
                neg1 = np.full(O_CORE, -1.0, np.float16)
                xr[:, g * BLK:g * BLK + N_IN] = np.stack(
                    [s1, s1, s2, s2, q1, q2, one_i, one_i]
                )
                xr[:, g * BLK + N_IN:(g + 1) * BLK] = np.stack(
                    [u1, u2, u1, u2, neg1, neg1, -w1, -w2]
                )
            m["xr"] = xr
        else:
            lx = np.stack([x, x * x, np.ones_like(x)]).astype(np.float32)
            rt = np.empty((3 * G, O_CORE), dtype=np.float32)
            for g, a in enumerate(alphas):
                rt[3 * g + 0] = 2.0 * a * th
                rt[3 * g + 1] = -a
                rt[3 * g + 2] = -a * th * th
            m["lx"], m["rt"] = lx, rt

        blocks = []
        for g, chans in enumerate(groups):
            w = widths[g]
            rhs = np.zeros((N_IN, w), dtype=np.float64)
            coff = 0
            if 0 in chans:
                rhs[:, 0] = 1.0
                coff = 1
            conv_ch = [c for c in chans if c > 0]
            if conv_ch:
                rhs[:, coff:] = y[:, [c - 1 for c in conv_ch]] @ W64[conv_ch, :]
            blocks.append(
                rhs.reshape(KI, P, w).transpose(1, 0, 2).reshape(P, KI * w)
            )
        ydt = np.concatenate(blocks, axis=1)
        m["ydt"] = ydt.astype(np.float16 if fp16_ok else np.float32)
        wb0 = np.tile(W64[0].astype(np.float32), (P, 1))
        bt = np.tile(b64.astype(np.float32), (P, 1))
        if fp16_ok:
            m["wbb"] = np.concatenate([wb0, bt], axis=1)
        else:
            m["wb0"], m["bt"] = wb0, bt
        in_maps.append(m)
    return widths, fp16_ok, in_maps


def _run(inputs: dict, trace: bool = False):
    """Compile (cached), run on 8 cores, gather. Returns (output, results)."""
    from concourse.bass_utils import run_bass_kernel_spmd

    banded = _try_prepare_banded(
        inputs["context_x"], inputs["context_y"], inputs["t"],
        inputs["sigma"], inputs["W"], inputs["b"],
    )
    if banded is not None:
        in_maps, scatter, has_bias = banded
        key = ("banded", has_bias)
        if key not in _BASS_CACHE:
            _BASS_CACHE[key] = _build_banded(has_bias)
        nc = _BASS_CACHE[key]
        res = run_bass_kernel_spmd(nc, in_maps, list(range(N_CORES)),
                                   trace=trace)
        out = np.empty((B, N_OUT, OUT_CH), dtype=np.float32)
        for core in range(N_CORES):
            bidx = core // 2
            r = res.results[core]["out"]            # [128, 4*16]
            r = r.reshape(P, KJ, OUT_CH).transpose(1, 0, 2).reshape(O_CORE, OUT_CH)
            out[bidx, scatter[core], :] = r
        return out, res

    widths, fp16_ok, in_maps = _prepare_inputs(
        inputs["context_x"], inputs["context_y"], inputs["t"],
        inputs["sigma"], inputs["W"], inputs["b"],
    )
    key = (widths, fp16_ok)
    if key not in _BASS_CACHE:
        _BASS_CACHE[key] = (_build_fp16_raw if fp16_ok else _build_fp32)(widths)
    nc = _BASS_CACHE[key]

    res = run_bass_kernel_spmd(nc, in_maps, list(range(N_CORES)), trace=trace)

    out = np.empty((B, N_OUT, OUT_CH), dtype=np.float32)
    for core in range(N_CORES):
        bidx, half = core // 2, core % 2
        out[bidx, half * O_CORE:(half + 1) * O_CORE, :] = res.results[core]["out"]
    return out, res


def kernel(**inputs) -> np.ndarray:
    out, _ = _run(inputs, trace=False)
    return out


# revision 12
# speedup vs baseline: 1.2422x; 1.0756x over previous
"""ConvDeepSet kernel for Trainium2 (8 NeuronCores, Bass/Tile).

Math (per batch b, target point o, channel c):
    agg[o,c] = sum_i yd[i,c] * exp(-alpha_c * (x_i - t_o)^2)      yd = [1 | y]
    out[o,:] = [agg0, agg1/(agg0+eps), ...] @ W + b

Fast path (banded, all sigmas equal): targets are sorted per batch on the
host; each 128-target block only interacts with a 512-wide window of the
sorted context points (RBF length scale 0.1 => weights beyond the window
are < 1e-8; the host validates the truncation error exactly in float64 and
falls back to the dense kernel if it exceeds tolerance).

Per core (batch b, half h of the sorted targets), per block k (128 targets):
  - exponent: -a(x-t)^2 = s*u - s^2/2 - u^2/2 via a rank-8 fp16 matmul of
    host-prepared 2-way fp16 splits (exact to ~1e-4).  The 4 window chunks
    are packed in the PE array with row-group tiling (contract dim 8 -> 4
    concurrent matmuls at tile_position (32c, 0)).
  - exp on ScalarE per block: PSUM [128,512] -> SBUF fp16.
  - aggregation: 4 matmuls per block, E chunk stationary, ydt (W folded in
    on the host) moving; all blocks accumulate into ONE PSUM bank
    pacc[128, 4*17] so the epilogue is 4 wide DVE ops instead of 16:
      recip = 1/den (strided), t1 = W0*den (broadcast APs),
      tmp = conv*recip, out = tmp + t1.   (eps dropped: den >> eps,
    validated on the host.)
  - single merged input DMA per queue and a single [128, 64] output DMA;
    the host un-sorts the targets when gathering.

Dense fp16 and fp32 variants are kept as fallbacks for inputs where the
banded assumptions (single sigma group, fp16-safe ranges, small truncation
error) do not hold.
"""

import numpy as np

B, N_IN, N_OUT = 4, 1024, 1024
IN_CH_RAW, OUT_CH = 7, 16
IN_CH = IN_CH_RAW + 1
N_CORES = 8
O_CORE = N_OUT // 2          # 512 target points per core
P = 128                      # partitions
KI = N_IN // P               # 8 contraction chunks (dense path)
NPAIR = KI // 2              # exp processed in chunk pairs (dense path)
KJ = O_CORE // P             # 4 output chunks
KEXP = 8                     # rows of the split-product exponent matmul
EPS = 1e-8

# banded-path geometry
WCTX = 512                   # context window per 128-target block
CH = WCTX // P               # 4 window chunks
BLKS = KJ                    # 4 target blocks per core
XRB = WCTX + P               # xr2 cols per block: 512 lhs + 128 rhs

_BASS_CACHE: dict = {}


def _build_banded(has_bias: bool):
    """Banded fp16 kernel: sorted targets, 512-wide context windows."""
    import concourse.bass as bass
    from concourse import mybir

    f32 = mybir.dt.float32
    f16 = mybir.dt.float16
    u8 = mybir.dt.uint8
    Exp = mybir.ActivationFunctionType.Exp
    Mult = mybir.AluOpType.mult

    # byte layout of the merged gpsimd DMA: w4 f32 | (bt f32) | ydt f16
    W4B = KJ * OUT_CH * 4                 # 256
    BTB = KJ * OUT_CH * 4 if has_bias else 0
    YDTB = BLKS * CH * 17 * 2             # 544
    NB = W4B + BTB + YDTB

    nc = bass.Bass("TRN2", target_bir_lowering=False, debug=False)

    # xr2 is split across the two HWDGE queues (SP + ACT): partitions 0-7
    # map to only 2 SDMA engines, so one 40KB transfer is engine-bound
    # (~1.2us); two 20KB transfers on separate rings halve that and let
    # block 0 start ~0.9us earlier.
    HB = BLKS * XRB // 2
    xr2a_d = nc.dram_tensor("xr2a", [KEXP, HB], f16, kind="ExternalInput")
    xr2b_d = nc.dram_tensor("xr2b", [KEXP, HB], f16, kind="ExternalInput")
    ydtw_d = nc.dram_tensor("ydtw", [P, NB], u8, kind="ExternalInput")
    out_d = nc.dram_tensor("out", [P, KJ * OUT_CH], f32, kind="ExternalOutput")

    xr2 = nc.alloc_sbuf_tensor("xr2_sb", [KEXP, BLKS * XRB], f16).ap()
    big = nc.alloc_sbuf_tensor("big_sb", [P, NB], u8).ap()
    w4 = big[:, 0:W4B].bitcast(f32)                       # [128, 64]
    bt = big[:, W4B:W4B + BTB].bitcast(f32) if has_bias else None
    ydt = big[:, W4B + BTB:NB].bitcast(f16)               # [128, 272]
    warm_w = nc.alloc_sbuf_tensor("warm_w_sb", [KEXP, 512], f16).ap()
    warm = nc.alloc_sbuf_tensor("warm_sb", [1, 1], f32).ap()
    E = [nc.alloc_sbuf_tensor(f"e_sb{k}", [P, WCTX], f16).ap()
         for k in range(BLKS)]
    rec = nc.alloc_sbuf_tensor("rec_sb", [P, KJ], f32).ap()
    t1 = nc.alloc_sbuf_tensor("t1_sb", [P, KJ * OUT_CH], f32).ap()
    tmp = nc.alloc_sbuf_tensor("tmp_sb", [P, KJ * OUT_CH], f32).ap()
    o_all = nc.alloc_sbuf_tensor("o_all_sb", [P, KJ * OUT_CH], f32).ap()

    A = [nc.alloc_psum_tensor(f"a_ps{i}", [P, WCTX], f32).ap()
         for i in range(2)]
    pacc = nc.alloc_psum_tensor("pacc_ps", [P, KJ * 17], f32).ap()
    warm_ps = nc.alloc_psum_tensor("warm_ps", [KEXP, 512], f32).ap()

    pacc3 = pacc.rearrange("p (k c) -> p k c", k=KJ)      # [128, 4, 17]
    den = pacc3[:, :, 0]                                  # [128, 4] strided
    conv3 = pacc3[:, :, 1:17]                             # [128, 4, 16]
    den_b = den.unsqueeze(2).broadcast_to([P, KJ, OUT_CH])
    rec_b = rec.unsqueeze(2).broadcast_to([P, KJ, OUT_CH])
    w4_3 = w4.rearrange("p (k c) -> p k c", k=KJ)
    t1_3 = t1.rearrange("p (k c) -> p k c", k=KJ)
    tmp_3 = tmp.rearrange("p (k c) -> p k c", k=KJ)

    with (
        nc.Block() as block,
        nc.semaphore("dsem_a") as dsem_a,   # sync-queue input DMA (xr2a)
        nc.semaphore("dsem_b") as dsem_b,   # scalar-queue input DMA (xr2b)
        nc.semaphore("dsem_g") as dsem_g,   # gpsimd-queue input DMA (ydtw)
        nc.semaphore("psem") as psem,       # PE exponent blocks done
        nc.semaphore("asem") as asem,       # ACT exp blocks done
        nc.semaphore("gsem") as gsem,       # PE aggregation done
        nc.semaphore("vv") as vv,           # DVE same-engine pipeline sync
        nc.semaphore("vsem") as vsem,       # DVE epilogue done
        nc.semaphore("osem") as osem,       # out DMA done (never waited on;
                                            # the Block-exit DRAIN on SP
                                            # blocks until DGE queues empty)
    ):
        @block.sync
        def _(sync):
            sync.dma_start(out=xr2[:, 0:HB], in_=xr2a_d[:]).then_inc(dsem_a, 16)
            sync.wait_ge(vsem, 1)
            sync.dma_start(out=out_d[:], in_=o_all[:]).then_inc(osem, 16)

        @block.gpsimd
        def _(gpsimd):
            gpsimd.dma_start(out=big[:], in_=ydtw_d[:]).then_inc(dsem_g, 16)

        @block.tensor
        def _(tensor):
            # HAM warmup while the input DMAs are in flight (uninitialized
            # operands; results discarded)
            for _w in range(3):
                tensor.matmul(warm_ps[:], warm_w[:, 0:KEXP], warm_w[:],
                              start=True, stop=True)
            tensor.wait_ge(dsem_a, 16)
            for k in range(BLKS):
                if k == 2:
                    tensor.wait_ge(dsem_b, 16)
                if k >= 2:
                    tensor.wait_ge(asem, k - 1)   # A[k%2] free again
                base = k * XRB
                for c in range(CH):
                    mm = tensor.matmul(
                        A[k % 2][:, c * P:(c + 1) * P],
                        xr2[:, base + c * P:base + (c + 1) * P],
                        xr2[:, base + WCTX:base + XRB],
                        start=True,
                        stop=True,
                    )
                    if c == CH - 1:
                        mm.then_inc(psem, 1)
            tensor.wait_ge(dsem_g, 16)
            for k in range(BLKS):
                tensor.wait_ge(asem, k + 1)
                for c in range(CH):
                    mm = tensor.matmul(
                        pacc[:, k * 17:(k + 1) * 17],
                        E[k][:, c * P:(c + 1) * P],
                        ydt[:, 17 * (CH * k + c):17 * (CH * k + c + 1)],
                        start=(c == 0),
                        stop=(c == CH - 1),
                        skip_group_check=True,
                    )
                    if k == BLKS - 1 and c == CH - 1:
                        mm.then_inc(gsem, 1)

        @block.scalar
        def _(scalar):
            # touch Exp before the pipeline needs it: loads the ACT table
            # while the input DMAs are still in flight
            scalar.activation(warm[:], nc.const_aps.tensor(0.0, (1, 1)), Exp)
            scalar.dma_start(out=xr2[:, HB:], in_=xr2b_d[:]).then_inc(dsem_b, 16)
            for k in range(BLKS):
                scalar.wait_ge(psem, k + 1)
                scalar.activation(E[k][:], A[k % 2][:], Exp).then_inc(asem, 1)

        @block.vector
        def _(vector):
            vector.wait_ge(dsem_g, 16)   # w4 resident
            vector.wait_ge(gsem, 1)      # pacc complete
            vector.reciprocal(rec[:], den).then_inc(vv, 1)
            vector.tensor_tensor(t1_3, w4_3, den_b, Mult).then_inc(vv, 1)
            vector.wait_ge(vv, 2)
            vector.tensor_tensor(tmp_3, conv3, rec_b, Mult).then_inc(vv, 1)
            vector.wait_ge(vv, 3)
            if has_bias:
                vector.scalar_tensor_tensor(
                    o_all[:], tmp[:], 1.0, t1[:],
                    op0=Mult, op1=mybir.AluOpType.add,
                ).then_inc(vv, 1)
                vector.wait_ge(vv, 4)
                vector.tensor_add(o_all[:], o_all[:], bt[:]).then_inc(vsem, 1)
            else:
                vector.tensor_add(o_all[:], tmp[:], t1[:]).then_inc(vsem, 1)

    return nc


def _build_fp16_raw(widths):
    """Dense fallback: raw-Bass fp16 pipeline (no banding)."""
    import concourse.bass as bass
    from concourse import mybir

    f32 = mybir.dt.float32
    f16 = mybir.dt.float16
    G = len(widths)
    wtot = sum(widths)
    offs = np.cumsum([0] + list(widths))
    npair = NPAIR * G
    Exp = mybir.ActivationFunctionType.Exp

    nc = bass.Bass("TRN2", target_bir_lowering=False, debug=False)

    BLK = N_IN + O_CORE
    xr_d = nc.dram_tensor("xr", [KEXP, G * BLK], f16, kind="ExternalInput")
    ydt_d = nc.dram_tensor("ydt", [P, KI * wtot], f16, kind="ExternalInput")
    wbb_d = nc.dram_tensor("wbb", [P, 2 * OUT_CH], f32, kind="ExternalInput")
    out_d = nc.dram_tensor("out", [O_CORE, OUT_CH], f32, kind="ExternalOutput")

    xr = nc.alloc_sbuf_tensor("xr_sb", [KEXP, G * BLK], f16).ap()
    lxh = [xr[:, g * BLK:g * BLK + N_IN] for g in range(G)]
    rth = [xr[:, g * BLK + N_IN:(g + 1) * BLK] for g in range(G)]
    ydt = nc.alloc_sbuf_tensor("ydt_sb", [P, KI * wtot], f16).ap()
    wbb = nc.alloc_sbuf_tensor("wbb_sb", [P, 2 * OUT_CH], f32).ap()
    wb0 = wbb[:, :OUT_CH]
    bt = wbb[:, OUT_CH:]
    warm = nc.alloc_sbuf_tensor("warm_sb", [1, 1], f32).ap()
    E = [nc.alloc_sbuf_tensor(f"e_sb{q}", [P, 2 * O_CORE], f16).ap()
         for q in range(npair)]
    denp = [nc.alloc_sbuf_tensor(f"denp_sb{kj}", [P, 1], f32).ap()
            for kj in range(KJ)]
    recip = [nc.alloc_sbuf_tensor(f"recip_sb{kj}", [P, 1], f32).ap()
             for kj in range(KJ)]
    t1 = [nc.alloc_sbuf_tensor(f"t1_sb{kj}", [P, OUT_CH], f32).ap()
          for kj in range(KJ)]
    o_all = nc.alloc_sbuf_tensor("o_all_sb", [P, KJ * OUT_CH], f32).ap()
    o_sb = [o_all[:, kj * OUT_CH:(kj + 1) * OUT_CH] for kj in range(KJ)]
    A = [nc.alloc_psum_tensor(f"a_ps{i}", [P, 2 * O_CORE], f32).ap()
         for i in range(2)]
    pacc = [nc.alloc_psum_tensor(f"pacc{kj}", [P, 1 + OUT_CH], f32).ap()
            for kj in range(KJ)]

    with (
        nc.Block() as block,
        nc.semaphore("dsem_s") as dsem_s,
        nc.semaphore("dsem_g") as dsem_g,
        nc.semaphore("psem") as psem,
        nc.semaphore("asem") as asem,
        nc.semaphore("gsem") as gsem,
        nc.semaphore("vsem") as vsem,
        nc.semaphore("vv") as vv,
        nc.semaphore("osem") as osem,
    ):
        @block.sync
        def _(sync):
            sync.dma_start(out=xr[:], in_=xr_d[:]).then_inc(dsem_s, 16)
            for kj in range(KJ):
                sync.wait_ge(vsem, kj + 1)
                sync.dma_start(
                    out=out_d[kj * P:(kj + 1) * P, :], in_=o_sb[kj][:]
                ).then_inc(osem, 16)

        @block.gpsimd
        def _(gpsimd):
            gpsimd.dma_start(out=ydt[:], in_=ydt_d[:]).then_inc(dsem_g, 16)
            gpsimd.dma_start(out=wbb[:], in_=wbb_d[:]).then_inc(dsem_g, 16)

        @block.tensor
        def _(tensor):
            tensor.wait_ge(dsem_s, 16)
            for q in range(npair):
                g, p = divmod(q, NPAIR)
                if q >= 2:
                    tensor.wait_ge(asem, q - 1)
                for half in range(2):
                    ki = 2 * p + half
                    tensor.matmul(
                        A[q % 2][:, half * O_CORE:(half + 1) * O_CORE],
                        lxh[g][:, ki * P:(ki + 1) * P],
                        rth[g][:],
                        start=True,
                        stop=True,
                    ).then_inc(psem, 1)
            tensor.wait_ge(dsem_g, 32)
            n_mm = G * KI
            for kj in range(KJ):
                for g in range(G):
                    w = widths[g]
                    coff = 0 if g == 0 else 1
                    for ki in range(KI):
                        idx = g * KI + ki
                        q = g * NPAIR + ki // 2
                        if kj == 0 and ki % 2 == 0:
                            tensor.wait_ge(asem, q + 1)
                        rhs = ydt[:, KI * offs[g] + ki * w:
                                  KI * offs[g] + (ki + 1) * w]
                        lhs = E[q][:, (ki % 2) * O_CORE + kj * P:
                                   (ki % 2) * O_CORE + (kj + 1) * P]
                        mm = tensor.matmul(
                            pacc[kj][:, coff:coff + w],
                            lhs,
                            rhs,
                            start=(idx == 0),
                            stop=(idx == n_mm - 1),
                            skip_group_check=(G > 1),
                        )
                        if idx == n_mm - 1:
                            mm.then_inc(gsem, 1)

        @block.scalar
        def _(scalar):
            scalar.activation(warm[:], nc.const_aps.tensor(0.0, (1, 1)), Exp)
            for q in range(npair):
                scalar.wait_ge(psem, 2 * (q + 1))
                scalar.activation(E[q][:], A[q % 2][:], Exp).then_inc(asem, 1)

        @block.vector
        def _(vector):
            vector.wait_ge(dsem_g, 32)
            for kj in range(KJ):
                vector.wait_ge(gsem, kj + 1)
                vector.tensor_scalar_add(
                    denp[kj][:], pacc[kj][:, 0:1], EPS
                ).then_inc(vv, 1)
                vector.wait_ge(vv, 3 * kj + 1)
                vector.scalar_tensor_tensor(
                    t1[kj][:], wb0[:], denp[kj][:], bt[:],
                    op0=mybir.AluOpType.mult, op1=mybir.AluOpType.add,
                ).then_inc(vv, 1)
                vector.reciprocal(recip[kj][:], denp[kj][:]).then_inc(vv, 1)
                vector.wait_ge(vv, 3 * kj + 3)
                vector.scalar_tensor_tensor(
                    o_sb[kj][:], pacc[kj][:, 1:1 + OUT_CH], recip[kj][:], t1[kj][:],
                    op0=mybir.AluOpType.mult, op1=mybir.AluOpType.add,
                ).then_inc(vsem, 1)

    return nc


def _build_fp32(widths):
    """Fallback: fp32 rank-3 exponent matmul + fp32 aggregation (slower,
    used only when fp16 split values would overflow)."""
    import concourse.bacc as bacc
    import concourse.tile as tile
    from concourse import mybir

    f32 = mybir.dt.float32
    G = len(widths)
    wtot = sum(widths)
    offs = np.cumsum([0] + list(widths))

    nc = bacc.Bacc("TRN2", target_bir_lowering=False, debug=False)

    lx_d = nc.dram_tensor("lx", [3, N_IN], f32, kind="ExternalInput")
    rt_d = nc.dram_tensor("rt", [3 * G, O_CORE], f32, kind="ExternalInput")
    ydt_d = nc.dram_tensor("ydt", [P, KI * wtot], f32, kind="ExternalInput")
    wb0_d = nc.dram_tensor("wb0", [P, OUT_CH], f32, kind="ExternalInput")
    bt_d = nc.dram_tensor("bt", [P, OUT_CH], f32, kind="ExternalInput")
    out_d = nc.dram_tensor("out", [O_CORE, OUT_CH], f32, kind="ExternalOutput")

    with tile.TileContext(nc) as tc:
        with (
            tc.tile_pool(name="const", bufs=1) as cpool,
            tc.tile_pool(name="epool", bufs=1) as epool,
            tc.tile_pool(name="small", bufs=2) as spool,
            tc.tile_pool(name="outp", bufs=2) as opool,
            tc.tile_pool(name="apsum", bufs=3, space="PSUM") as apsum,
            tc.tile_pool(name="ppsum", bufs=1, space="PSUM") as ppsum,
        ):
            lx = cpool.tile([3, N_IN], f32, tag="lx")
            nc.sync.dma_start(lx[:], lx_d[:])
            rt = cpool.tile([3 * G, O_CORE], f32, tag="rt")
            nc.scalar.dma_start(rt[:], rt_d[:])
            ydt = cpool.tile([P, KI * wtot], f32, tag="ydt")
            nc.gpsimd.dma_start(ydt[:], ydt_d[:])
            wb0 = cpool.tile([P, OUT_CH], f32, tag="wb0")
            nc.gpsimd.dma_start(wb0[:], wb0_d[:])
            bt = cpool.tile([P, OUT_CH], f32, tag="bt")
            nc.gpsimd.dma_start(bt[:], bt_d[:])

            E = {}
            for g in range(G):
                for ki in range(KI):
                    a_ps = apsum.tile([P, O_CORE], f32, tag="A", name="a_ps")
                    nc.tensor.matmul(
                        a_ps[:],
                        lx[:, ki * P:(ki + 1) * P],
                        rt[3 * g:3 * g + 3, :],
                        start=True,
                        stop=True,
                    )
                    e = epool.tile([P, O_CORE], f32, tag=f"E{g}_{ki}", name="e")
                    nc.scalar.activation(
                        e[:], a_ps[:], mybir.ActivationFunctionType.Exp
                    )
                    E[(g, ki)] = e

            pacc = [
                ppsum.tile([P, 1 + OUT_CH], f32, tag=f"P{kj}", name=f"pacc{kj}")
                for kj in range(KJ)
            ]
            n_mm = G * KI
            for kj in range(KJ):
                for g in range(G):
                    w = widths[g]
                    coff = 0 if g == 0 else 1
                    for ki in range(KI):
                        idx = g * KI + ki
                        rhs = ydt[:, KI * offs[g] + ki * w: KI * offs[g] + (ki + 1) * w]
                        nc.tensor.matmul(
                            pacc[kj][:, coff:coff + w],
                            E[(g, ki)][:, kj * P:(kj + 1) * P],
                            rhs,
                            start=(idx == 0),
                            stop=(idx == n_mm - 1),
                            skip_group_check=(G > 1),
                        )

                denp = spool.tile([P, 1], f32, tag="denp", name="denp")
                nc.vector.tensor_scalar_add(denp[:], pacc[kj][:, 0:1], EPS)
                recip = spool.tile([P, 1], f32, tag="recip", name="recip")
                nc.vector.reciprocal(recip[:], denp[:])
                t1 = spool.tile([P, OUT_CH], f32, tag="t1", name="t1")
                nc.vector.scalar_tensor_tensor(
                    t1[:], wb0[:], denp[:], bt[:],
                    op0=mybir.AluOpType.mult, op1=mybir.AluOpType.add,
                )
                o_sb = opool.tile([P, OUT_CH], f32, tag="osb", name="o_sb")
                nc.vector.scalar_tensor_tensor(
                    o_sb[:], pacc[kj][:, 1:1 + OUT_CH], recip[:], t1[:],
                    op0=mybir.AluOpType.mult, op1=mybir.AluOpType.add,
                )
                nc.sync.dma_start(out_d[kj * P:(kj + 1) * P, :], o_sb[:])

    nc.compile()
    return nc


def _split2_f16(v):
    """2-way fp16 split: v ~= h1 + h2 with each half exactly fp16."""
    v = v.astype(np.float32)
    h1 = v.astype(np.float16)
    h2 = (v - h1.astype(np.float32)).astype(np.float16)
    return h1, h2


def _sigma_groups(sigma):
    sigma = np.asarray(sigma, dtype=np.float32)
    uniq = []
    for c in range(IN_CH):
        if sigma[c] not in uniq:
            uniq.append(sigma[c])
    uniq.sort(key=lambda s: (s != sigma[0]))  # channel-0 group first
    groups = [[c for c in range(IN_CH) if sigma[c] == s] for s in uniq]
    alphas = [0.5 / np.exp(2.0 * np.float64(s)) for s in uniq]
    widths = tuple((1 + OUT_CH) if 0 in g else OUT_CH for g in groups)
    return groups, alphas, widths


def _try_prepare_banded(context_x, context_y, t, sigma, W, b):
    """Banded host prep.  Returns (in_maps, scatter, has_bias) or None if
    the banded assumptions fail on this input (multi-sigma, fp16-unsafe
    ranges, or truncation error above tolerance -- all checked exactly)."""
    groups, alphas, widths = _sigma_groups(sigma)
    if len(groups) != 1:
        return None
    a = float(alphas[0])
    if not np.isfinite(a):
        return None
    xmax = max(
        float(np.abs(np.asarray(context_x)).max()),
        float(np.abs(np.asarray(t)).max()),
        1.0,
    )
    if not (a * xmax * xmax < 3e4):
        return None

    W64 = np.asarray(W, dtype=np.float64)
    b64 = np.asarray(b, dtype=np.float64)
    has_bias = bool(np.any(b64 != 0.0))

    x_all = np.asarray(context_x, np.float64)[:, :, 0]
    t_all = np.asarray(t, np.float64)[:, :, 0]
    y_all = np.asarray(context_y, np.float64)

    # validate truncation exactly (float64) and gather the windows
    in_maps = [None] * N_CORES
    scatter = [None] * N_CORES
    err_max = 0.0
    den_min = np.inf
    for bidx in range(B):
        x = x_all[bidx]
        xs_idx = np.argsort(x, kind="stable")
        xs = x[xs_idx]
        ts_idx = np.argsort(t_all[bidx], kind="stable")
        ts = t_all[bidx][ts_idx]
        yd = np.empty((N_IN, 1 + OUT_CH))
        yd[:, 0] = 1.0
        yd[:, 1:] = y_all[bidx] @ W64[1:, :]

        # exact reference aggregation per block + banded version
        for half in range(2):
            core = bidx * 2 + half
            xr2 = np.empty((KEXP, BLKS * XRB), dtype=np.float16)
            ydtb = np.empty((P, BLKS * CH * 17), dtype=np.float16)
            for k in range(BLKS):
                pos = half * O_CORE + k * P
                tb = ts[pos:pos + P]
                mid = 0.5 * (tb[0] + tb[-1])
                cpos = np.searchsorted(xs, mid)
                lo = int(np.clip(cpos - WCTX // 2, 0, N_IN - WCTX))
                w_idx = xs_idx[lo:lo + WCTX]
                xw = x[w_idx]

                # truncation error (exact, float64)
                excl = np.concatenate([xs_idx[:lo], xs_idx[lo + WCTX:]])
                Ee = np.exp(-a * (x[excl][:, None] - tb[None, :]) ** 2)
                d_agg = Ee.T @ yd[excl]                  # (128, 17)
                Ew = np.exp(-a * (xw[:, None] - tb[None, :]) ** 2)
                den_w = Ew.T @ yd[w_idx, 0]              # (128,)
                den_min = min(den_min, float(den_w.min()))
                # |d out| <= |d den|*|W0| + (|d conv| + |conv/den|*|d den|)/den
                conv_w = Ew.T @ yd[w_idx, 1:]
                ratio = np.abs(conv_w) / den_w[:, None]
                e_out = (np.abs(d_agg[:, 0:1]) * np.abs(W64[0]) +
                         (np.abs(d_agg[:, 1:]) + ratio * d_agg[:, 0:1]) / den_w[:, None])
                err_max = max(err_max, float(e_out.max()))

                # device data
                r = np.sqrt(2.0 * a)
                s1, s2 = _split2_f16(r * xw)
                q1, q2 = _split2_f16(0.5 * (r * xw) ** 2)
                u1, u2 = _split2_f16(r * tb)
                v1, v2 = _split2_f16(0.5 * (r * tb) ** 2)
                one_i = np.ones(WCTX, np.float16)
                neg1 = np.full(P, -1.0, np.float16)
                base = k * XRB
                xr2[:, base:base + WCTX] = np.stack(
                    [s1, s1, s2, s2, q1, q2, one_i, one_i])
                xr2[:, base + WCTX:base + XRB] = np.stack(
                    [u1, u2, u1, u2, neg1, neg1, -v1, -v2])
                for c in range(CH):
                    sl = slice(c * P, (c + 1) * P)
                    ydtb[:, 17 * (CH * k + c):17 * (CH * k + c + 1)] = \
                        yd[w_idx[sl]].astype(np.float16)

            w4 = np.tile(W64[0].astype(np.float32), (P, KJ))
            parts = [w4.view(np.uint8).reshape(P, -1)]
            if has_bias:
                btile = np.tile(b64.astype(np.float32), (P, KJ))
                parts.append(btile.view(np.uint8).reshape(P, -1))
            parts.append(ydtb.view(np.uint8).reshape(P, -1))
            ydtw = np.concatenate(parts, axis=1)

            hb = BLKS * XRB // 2
            in_maps[core] = {"xr2a": np.ascontiguousarray(xr2[:, :hb]),
                             "xr2b": np.ascontiguousarray(xr2[:, hb:]),
                             "ydtw": ydtw}
            scatter[core] = ts_idx[half * O_CORE:(half + 1) * O_CORE]

    if err_max > 2.5e-3 or den_min < 1e-6:
        return None
    return in_maps, scatter, has_bias


def _prepare_inputs(context_x, context_y, t, sigma, W, b):
    """Dense host prep: group channels by sigma, fold W, per-core inputs."""
    sigma = np.asarray(sigma, dtype=np.float32)
    W64 = np.asarray(W, dtype=np.float64)
    b64 = np.asarray(b, dtype=np.float64)

    groups, alphas, widths = _sigma_groups(sigma)
    G = len(groups)

    xmax = max(
        float(np.abs(np.asarray(context_x)).max()),
        float(np.abs(np.asarray(t)).max()),
        1.0,
    )
    fp16_ok = all(a * xmax * xmax < 3e4 and np.isfinite(a) for a in alphas)

    in_maps = []
    for core in range(N_CORES):
        bidx, half = core // 2, core % 2
        x = np.asarray(context_x[bidx, :, 0], dtype=np.float64)
        th = np.asarray(t[bidx, half * O_CORE:(half + 1) * O_CORE, 0],
                        dtype=np.float64)
        y = np.asarray(context_y[bidx], dtype=np.float64)

        m = {}
        if fp16_ok:
            BLK = N_IN + O_CORE
            xr = np.empty((KEXP, G * BLK), dtype=np.float16)
            for g, a in enumerate(alphas):
                r = np.sqrt(2.0 * a)
                s1, s2 = _split2_f16(r * x)
                u1, u2 = _split2_f16(r * th)
                q1, q2 = _split2_f16(0.5 * (r * x) ** 2)
                w1, w2 = _split2_f16(0.5 * (r * th) ** 2)
                one_i = np.ones(N_IN, np.float16)
                neg1 = np.full(O_CORE, -1.0, np.float16)
                xr[:, g * BLK:g * BLK + N_IN] = np.stack(
                    [s1, s1, s2, s2, q1, q2, one_i, one_i]
                )
                xr[:, g * BLK + N_IN:(g + 1) * BLK] = np.stack(
                    [u1, u2, u1, u2, neg1, neg1, -w1, -w2]
                )
            m["xr"] = xr
        else:
            lx = np.stack([x, x * x, np.ones_like(x)]).astype(np.float32)
            rt = np.empty((3 * G, O_CORE), dtype=np.float32)
            for g, a in enumerate(alphas):
                rt[3 * g + 0] = 2.0 * a * th
                rt[3 * g + 1] = -a
                rt[3 * g + 2] = -a * th * th
            m["lx"], m["rt"] = lx, rt

        blocks = []
        for g, chans in enumerate(groups):
            w = widths[g]
            rhs = np.zeros((N_IN, w), dtype=np.float64)
            coff = 0
            if 0 in chans:
                rhs[:, 0] = 1.0
                coff = 1
            conv_ch = [c for c in chans if c > 0]
            if conv_ch:
                rhs[:, coff:] = y[:, [c - 1 for c in conv_ch]] @ W64[conv_ch, :]
            blocks.append(
                rhs.reshape(KI, P, w).transpose(1, 0, 2).reshape(P, KI * w)
            )
        ydt = np.concatenate(blocks, axis=1)
        m["ydt"] = ydt.astype(np.float16 if fp16_ok else np.float32)
        wb0 = np.tile(W64[0].astype(np.float32), (P, 1))
        bt = np.tile(b64.astype(np.float32), (P, 1))
        if fp16_ok:
            m["wbb"] = np.concatenate([wb0, bt], axis=1)
        else:
            m["wb0"], m["bt"] = wb0, bt
        in_maps.append(m)
    return widths, fp16_ok, in_maps


def _run(inputs: dict, trace: bool = False):
    """Compile (cached), run on 8 cores, gather. Returns (output, results)."""
    from concourse.bass_utils import run_bass_kernel_spmd

    banded = _try_prepare_banded(
        inputs["context_x"], inputs["context_y"], inputs["t"],
        inputs["sigma"], inputs["W"], inputs["b"],
    )
    if banded is not None:
        in_maps, scatter, has_bias = banded
        key = ("banded", has_bias)
        if key not in _BASS_CACHE:
            _BASS_CACHE[key] = _build_banded(has_bias)
        nc = _BASS_CACHE[key]
        res = run_bass_kernel_spmd(nc, in_maps, list(range(N_CORES)),
                                   trace=trace)
        out = np.empty((B, N_OUT, OUT_CH), dtype=np.float32)
        for core in range(N_CORES):
            bidx = core // 2
            r = res.results[core]["out"]            # [128, 4*16]
            r = r.reshape(P, KJ, OUT_CH).transpose(1, 0, 2).reshape(O_CORE, OUT_CH)
            out[bidx, scatter[core], :] = r
        return out, res

    widths, fp16_ok, in_maps = _prepare_inputs(
        inputs["context_x"], inputs["context_y"], inputs["t"],
        inputs["sigma"], inputs["W"], inputs["b"],
    )
    key = (widths, fp16_ok)
    if key not in _BASS_CACHE:
        _BASS_CACHE[key] = (_build_fp16_raw if fp16_ok else _build_fp32)(widths)
    nc = _BASS_CACHE[key]

    res = run_bass_kernel_spmd(nc, in_maps, list(range(N_CORES)), trace=trace)

    out = np.empty((B, N_OUT, OUT_CH), dtype=np.float32)
    for core in range(N_CORES):
        bidx, half = core // 2, core % 2
        out[bidx, half * O_CORE:(half + 1) * O_CORE, :] = res.results[core]["out"]
    return out, res


def kernel(**inputs) -> np.ndarray:
    out, _ = _run(inputs, trace=False)
    return out
